# revision 5
# baseline (speedup 1.0000x reference)
"""Local (windowed) attention kernel for Trainium2, 8 NeuronCores.

Problem: q,k,v [2,16,4096,128] f32; window=256, look_backward=1, causal,
exact_windowsize. Each query window w (256 queries) attends to key windows
w-1 and w (512 keys) with a banded causal mask:
  prev-window keys (local j): keep where j >= i   (upper tri incl diag)
  own-window keys (local j):  keep where j <= i   (lower tri incl diag)

Sharding: merged batch*heads dim B=32 split across 8 cores (4 rows each).

The wall-clock of a warm run is dominated by host<->device transfer of the
I/O tensors, so the kernel is laid out to minimize bytes moved:
  - q,k are host-pre-transposed to [e, t] bf16 (QK^T needs no on-device
    transpose), v is sent natural [t, e] bf16 with a ones column appended
    per 128-key chunk ([t, 129] effectively), so each AV matmul produces
    both the output numerator and the softmax denominator in one PSUM tile.
  - softmax normalization happens on device (DVE reciprocal + per-partition
    scalar multiply) and the output is written in natural [t, e] layout as
    bf16, so the host does no divide/transpose and the output buffer (and
    its donated zero-init upload) is 4x smaller than an f32 transposed one.

Matmuls run in bf16 (inputs rounded on host); PSUM accumulation is f32.
exp() runs on the Scalar engine in f32 from PSUM.
"""
import numpy as np
import ml_dtypes
from concurrent.futures import ThreadPoolExecutor
from contextlib import ExitStack

import concourse.bacc as bacc
import concourse.mybir as mybir
from concourse import tile
from concourse.bass_utils import run_bass_kernel_spmd

F32 = mybir.dt.float32
BF16 = mybir.dt.bfloat16
AF = mybir.ActivationFunctionType
ALU = mybir.AluOpType

B, H, T, E = 2, 16, 4096, 128
WS = 256                 # window size (queries per window)
NW = T // WS             # 16 windows
NCORES = 8
U = (B * H) // NCORES    # 4 (b,h) rows per core
EA = E + 1               # v chunk width with ones column appended
SCALE = float(E) ** -0.5

_cached = {}


def _build_nc():
    nc = bacc.Bacc()
    qT_d = nc.declare_dram_parameter("qT", [U, E, T], BF16, isOutput=False)
    kT_d = nc.declare_dram_parameter("kT", [U, E, T], BF16, isOutput=False)
    va_d = nc.declare_dram_parameter("va", [U, T, EA], BF16, isOutput=False)
    out_d = nc.declare_dram_parameter("out", [U, T, E], BF16, isOutput=True)

    with tile.TileContext(nc) as tc, ExitStack() as ctx:
        big = ctx.enter_context(tc.tile_pool(name="big", bufs=2))
        epool = ctx.enter_context(tc.tile_pool(name="epool", bufs=6))
        opool = ctx.enter_context(tc.tile_pool(name="opool", bufs=4))
        rpool = ctx.enter_context(tc.tile_pool(name="rpool", bufs=4))
        ps_sc = ctx.enter_context(tc.tile_pool(name="ps_sc", bufs=3, space="PSUM"))
        ps_o = ctx.enter_context(tc.tile_pool(name="ps_o", bufs=4, space="PSUM"))

        for u in range(U):
            # per-u big loads
            # v_sb[p, EA*c + e] = va[128c+p, e]  (key chunks of 128 on
            # partitions; col 128 of each chunk is the appended 1.0)
            v_sb = big.tile([128, (T // 128) * EA], BF16, tag="v")
            nc.gpsimd.dma_start(v_sb[:].rearrange("p (c e) -> p c e", e=EA),
                                va_d[u].rearrange("(c p) e -> p c e", p=128))
            qT_sb = big.tile([E, T], BF16, tag="qT")
            nc.gpsimd.dma_start(qT_sb[:], qT_d[u])
            kT_sb = big.tile([E, T], BF16, tag="kT")
            nc.gpsimd.dma_start(kT_sb[:], kT_d[u])

            ebanks = {}   # (w, c) -> masked exp tile [128 keys, 512 queries]
            for w in range(NW):
                ncols = 2 * WS if w < NW - 1 else WS
                for c in range(2):
                    # scoresT bank: keys = window w half c (128 of them, on
                    # partitions), queries = windows w (cols 0:256) and w+1
                    # (cols 256:512)
                    sc = ps_sc.tile([128, 2 * WS], F32, tag="sc")
                    nc.tensor.matmul(
                        sc[:, 0:ncols],
                        lhsT=kT_sb[:, WS * w + 128 * c:WS * w + 128 * (c + 1)],
                        rhs=qT_sb[:, WS * w:WS * w + ncols],
                        start=True, stop=True)
                    eraw = epool.tile([128, 2 * WS], BF16, tag="eraw")
                    nc.scalar.activation(eraw[:, 0:ncols], sc[:, 0:ncols],
                                         AF.Exp, scale=SCALE)
                    et = epool.tile([128, 2 * WS], BF16, tag="et")
                    # own-window half for queries w: keep j<=i:
                    # iota = i - (128c+p) >= 0
                    nc.gpsimd.affine_select(
                        et[:, 0:WS], eraw[:, 0:WS], pattern=[[1, WS]],
                        base=-128 * c, channel_multiplier=-1,
                        compare_op=ALU.is_ge, fill=0.0)
                    if ncols == 2 * WS:
                        # prev-window half for queries w+1: keep j>=i:
                        # iota = (128c+p) - i >= 0
                        nc.gpsimd.affine_select(
                            et[:, WS:2 * WS], eraw[:, WS:2 * WS],
                            pattern=[[-1, WS]], base=128 * c,
                            channel_multiplier=1,
                            compare_op=ALU.is_ge, fill=0.0)
                    ebanks[(w, c)] = et

                # output for query window w: keys from windows w-1 and w.
                # Two 128-query chunks; each PSUM tile [128, 129] accumulates
                # numerator (cols 0:128) and denominator (col 128, from the
                # ones column of va) over the 4 (2 for w=0) key chunks.
                srcs = []
                if w > 0:
                    srcs += [(w - 1, 0, WS), (w - 1, 1, WS)]
                srcs += [(w, 0, 0), (w, 1, 0)]
                for qc in range(2):
                    op = ps_o.tile([128, EA], F32, tag="op")
                    for idx, (sw, cc, co) in enumerate(srcs):
                        et = ebanks[(sw, cc)]
                        vc0 = EA * (2 * sw + cc)
                        nc.tensor.matmul(
                            op[:],
                            lhsT=et[:, co + 128 * qc:co + 128 * qc + 128],
                            rhs=v_sb[:, vc0:vc0 + EA],
                            start=(idx == 0), stop=(idx == len(srcs) - 1))
                    r = rpool.tile([128, 1], F32)
                    nc.vector.reciprocal(r[:], op[:, E:EA])
                    out_sb = opool.tile([128, E], BF16)
                    nc.vector.tensor_scalar_mul(out_sb[:], op[:, 0:E], r[:])
                    t0 = WS * w + 128 * qc
                    nc.sync.dma_start(out_d[u][t0:t0 + 128, :], out_sb[:])
                if w >= 1:
                    ebanks.pop((w - 1, 0))
                    ebanks.pop((w - 1, 1))
    nc.finalize()
    return nc


def _prep_in_maps(q, k, v):
    """Host-side prep: shard rows across cores, pre-transpose q,k to [e,t]
    bf16, append the ones column to v. Returns run_bass_kernel_spmd in_maps."""
    qf = np.asarray(q).reshape(B * H, T, E)
    kf = np.asarray(k).reshape(B * H, T, E)
    vf = np.asarray(v).reshape(B * H, T, E)

    def _one(m):
        rows = slice(U * m, U * (m + 1))
        va = np.empty((U, T, EA), dtype=ml_dtypes.bfloat16)
        va[:, :, 0:E] = vf[rows]
        va[:, :, E] = 1.0
        return {
            "qT": qf[rows].transpose(0, 2, 1).astype(ml_dtypes.bfloat16),
            "kT": kf[rows].transpose(0, 2, 1).astype(ml_dtypes.bfloat16),
            "va": va,
        }

    with ThreadPoolExecutor(NCORES) as ex:
        return list(ex.map(_one, range(NCORES)))


def kernel(q, k, v):
    q = np.asarray(q); k = np.asarray(k); v = np.asarray(v)
    in_dt = q.dtype

    if "nc" not in _cached:
        _cached["nc"] = _build_nc()
    nc = _cached["nc"]

    in_maps = _prep_in_maps(q, k, v)
    try:
        res = run_bass_kernel_spmd(nc, in_maps, core_ids=list(range(NCORES)))
    except Exception:
        # one retry: the tunneled transport can fail transiently
        res = run_bass_kernel_spmd(nc, in_maps, core_ids=list(range(NCORES)))

    out = np.empty((B * H, T, E), dtype=np.float32)
    for m in range(NCORES):
        out[U * m:U * (m + 1)] = np.asarray(res.results[m]["out"],
                                            dtype=np.float32)
    return out.reshape(B, H, T, E).astype(in_dt, copy=False)
